# revision 1
# baseline (speedup 1.0000x reference)
"""Trainium2 Bass kernel for a 2-layer GAT (nn_GAT_70909910057105).

Strategy (8 NeuronCores, SPMD):
  - Core k owns target nodes [128k, 128k+128). Edges are bucketed by trg//128
    on the host (integer-only preprocessing), then sub-bucketed by src//256 so
    edge-feature rows can be gathered with int16 indices.
  - A bf16 DRAM "node table" holds per-node rows
    [h bf16 x1024 | a_src f32 x16 (bitcast) | a_tgt f32 x16 | pad] (1152 bf16).
    Per-edge source rows are fetched with dma_gather (2304B rows).
  - segment_sum becomes a PSUM-accumulated bf16 matmul with host-built one-hot
    masks: out[t, :] += mask_chunk.T @ (exp * h_src_chunk).
  - Per-edge target alphas come from a bf16 matmul maskT @ [at_hi | at_res]
    (hi/residual split keeps f32-level precision).
  - Edge-feature projection pe = (e_feats @ We.T).sum_per_head * a_e collapses
    to e_feats @ wesum (f32, computed on device), once for both layers.
  - One AllGather per layer rebuilds the replicated node table.
"""
import sys

for _p in ("/opt/trn_rl_repo", "/root/.axon_site/_ro/trn_rl_repo"):
    if _p not in sys.path:
        sys.path.insert(0, _p)

import numpy as np
import ml_dtypes
import concourse.bass as bass
import concourse.bacc as bacc
import concourse.tile as tile
from concourse import mybir
from concourse.bass_utils import run_bass_kernel_spmd
from concourse.masks import make_identity

F32 = mybir.dt.float32
BF16 = mybir.dt.bfloat16
I16 = mybir.dt.int16
NPBF = ml_dtypes.bfloat16

N, B, C, H, D = 1024, 4, 256, 4, 64
E = 32768
NC = 8
TPC = N // NC           # target nodes per core = 128
ROW = 1152              # bf16 elems: 1024 h | 32 (16 f32 a_src) | 32 (a_tgt) | 64 pad
AS_OFF = 1024           # bf16-elem offset of a_src f32 region
AT_OFF = 1056
NB_LOCAL = TPC * B      # 512 local (node, batch) rows
Q = 4                   # src quarters (int16 edge-feature indexing)
QROWS = (N // Q) * TPC  # 32768 rows per edge-feature shard quarter


# --------------------------------------------------------------------------
# host-side preprocessing (integer / layout ops only)
# --------------------------------------------------------------------------

def _pack_idx(vals: np.ndarray) -> np.ndarray:
    n = vals.shape[0]
    assert n % 16 == 0
    blk = vals.astype(np.int16).reshape(n // 16, 16).T
    return np.ascontiguousarray(np.tile(blk, (8, 1)))


def _prep(x, edge_features, src_idx, trg_idx,
          Wn1, We1, a_src1, a_tgt1, a_edge1,
          Wn2, We2, a_src2, a_tgt2, a_edge2):
    src = np.asarray(src_idx).astype(np.int64)
    trg = np.asarray(trg_idx).astype(np.int64)
    x = np.asarray(x, dtype=np.float32)
    ef = np.asarray(edge_features, dtype=np.float32)

    per_core = []
    bmax = 0
    for k in range(NC):
        eids = np.nonzero((trg // TPC) == k)[0]
        bks = [eids[(src[eids] // (N // Q)) == q] for q in range(Q)]
        per_core.append(bks)
        bmax = max(bmax, max(len(b) for b in bks))
    B_pad = ((bmax + 127) // 128) * 128
    E_pad = Q * B_pad

    xf = x.reshape(N * B, C)
    xT = np.ascontiguousarray(xf.T)

    def sb3(w, inner):
        return np.ascontiguousarray(w.reshape(2, 128, inner).transpose(1, 0, 2))

    def hsel(a_e):
        m = np.zeros((C, H), np.float32)
        for h in range(H):
            m[h * D:(h + 1) * D, h] = np.float32(a_e[h])
        return sb3(m, H)

    def ablk(a_s, a_t):
        m = np.zeros((C, 2 * H), np.float32)
        for h in range(H):
            m[h * D:(h + 1) * D, h] = np.asarray(a_s)[h]
            m[h * D:(h + 1) * D, H + h] = np.asarray(a_t)[h]
        return sb3(m, 2 * H)

    common = {
        "wn1hd": sb3(np.asarray(Wn1, np.float32), C),
        "wn2hd": sb3(np.asarray(Wn2, np.float32), C),
        "wn1cols": sb3(np.ascontiguousarray(np.asarray(Wn1, np.float32).T), C),
        "wn2cols": sb3(np.ascontiguousarray(np.asarray(Wn2, np.float32).T), C),
        "we1hd": sb3(np.asarray(We1, np.float32), C),
        "we2hd": sb3(np.asarray(We2, np.float32), C),
        "hsel1": hsel(np.asarray(a_edge1)),
        "hsel2": hsel(np.asarray(a_edge2)),
        "ablk1": ablk(a_src1, a_tgt1),
        "ablk2": ablk(a_src2, a_tgt2),
    }

    in_maps = []
    for k in range(NC):
        src_s = np.zeros(E_pad, np.int64)
        efi_s = np.zeros(E_pad, np.int64)
        mask = np.zeros((128, E_pad), np.float32)
        maskT = np.zeros((128, E_pad), np.float32)
        for q in range(Q):
            ids = per_core[k][q]
            s0 = q * B_pad
            src_s[s0:s0 + len(ids)] = src[ids]
            tl = trg[ids] - k * TPC
            efi_s[s0:s0 + len(ids)] = (src[ids] - q * (N // Q)) * TPC + tl
            for sslot, t in zip(range(s0, s0 + len(ids)), tl):
                mask[sslot % 128, (sslot // 128) * 128 + t] = 1.0
                maskT[t, (sslot // 128) * 128 + sslot % 128] = 1.0
        shard = np.ascontiguousarray(
            ef[:, k * TPC:(k + 1) * TPC, :]).reshape(N * TPC, C)
        m = dict(common)
        m.update({
            "ef": shard,
            "xT": np.ascontiguousarray(
                xT[:, k * NB_LOCAL:(k + 1) * NB_LOCAL]
            ).reshape(2, 128, NB_LOCAL).transpose(1, 0, 2).copy(),
            "isrc": _pack_idx(src_s),
            "ief": _pack_idx(efi_s),
            "mask": mask.astype(NPBF),
            "maskT": maskT.astype(NPBF),
        })
        in_maps.append(m)
    return in_maps, B_pad, E_pad, E_pad // 128


# --------------------------------------------------------------------------
# device program
# --------------------------------------------------------------------------

def _build(B_pad: int, debug: bool = False, stop_after: str = "full"):
    E_pad = Q * B_pad
    n_chunks = E_pad // 128
    n_super = E_pad // 512
    nc = bacc.Bacc("TRN2", target_bir_lowering=False, debug=False,
                   num_devices=NC)

    ef_in = nc.dram_tensor("ef", [Q * QROWS, C], F32, kind="ExternalInput")
    xT_in = nc.dram_tensor("xT", [128, 2, NB_LOCAL], F32, kind="ExternalInput")
    isrc_in = nc.dram_tensor("isrc", [128, E_pad // 16], I16, kind="ExternalInput")
    ief_in = nc.dram_tensor("ief", [128, E_pad // 16], I16, kind="ExternalInput")
    mask_in = nc.dram_tensor("mask", [128, E_pad], BF16, kind="ExternalInput")
    maskT_in = nc.dram_tensor("maskT", [128, E_pad], BF16, kind="ExternalInput")
    w_in = {
        nm: nc.dram_tensor(nm, [128, 2, inner], F32, kind="ExternalInput")
        for nm, inner in [
            ("wn1hd", C), ("wn2hd", C), ("wn1cols", C), ("wn2cols", C),
            ("we1hd", C), ("we2hd", C),
            ("hsel1", H), ("hsel2", H), ("ablk1", 2 * H), ("ablk2", 2 * H),
        ]
    }
    y_out = nc.dram_tensor("y", [128, B * C], F32, kind="ExternalOutput")
    dbg = {}
    if debug:
        for nm, shape, dt in [("dbg_x1", [128, B * C], F32),
                              ("dbg_pe", [128, n_chunks, 8], F32),
                              ("dbg_tbl", [N, ROW], BF16)]:
            dbg[nm] = nc.dram_tensor(nm, shape, dt, kind="ExternalOutput")

    from contextlib import ExitStack
    with tile.TileContext(nc) as tc:
        with ExitStack() as ctx:
            const = ctx.enter_context(tc.tile_pool(name="const", bufs=1))
            sb = ctx.enter_context(tc.tile_pool(name="sb", bufs=1))
            small = ctx.enter_context(tc.tile_pool(name="small", bufs=3))
            gpool = ctx.enter_context(tc.tile_pool(name="gpool", bufs=3))
            efpool = ctx.enter_context(tc.tile_pool(name="efpool", bufs=2))
            ps_small = ctx.enter_context(
                tc.tile_pool(name="ps_small", bufs=2, space="PSUM"))
            ps_t = ctx.enter_context(
                tc.tile_pool(name="ps_t", bufs=2, space="PSUM"))
            ps_out = ctx.enter_context(
                tc.tile_pool(name="ps_out", bufs=1, space="PSUM"))
            ps_den = ctx.enter_context(
                tc.tile_pool(name="ps_den", bufs=1, space="PSUM"))
            dram = ctx.enter_context(tc.tile_pool(name="dram", bufs=1, space="DRAM"))

            ident = const.tile([128, 128], F32)
            make_identity(nc, ident[:])
            zpad = const.tile([128, 16], BF16)
            nc.vector.memset(zpad[:], 0.0)

            w_sb = {}
            for nm, t in w_in.items():
                inner = t.shape[2]
                w_sb[nm] = const.tile([128, 2, inner], F32, name=f"w_{nm}",
                                      tag=f"w_{nm}")
                nc.sync.dma_start(out=w_sb[nm][:], in_=t[:])
            xT_sb = const.tile([128, 2, NB_LOCAL], F32)
            nc.sync.dma_start(out=xT_sb[:], in_=xT_in[:])
            isrc_t = const.tile([128, E_pad // 16], I16)
            nc.sync.dma_start(out=isrc_t[:], in_=isrc_in[:])
            ief_t = const.tile([128, E_pad // 16], I16)
            nc.sync.dma_start(out=ief_t[:], in_=ief_in[:])
            mask_sb = const.tile([128, E_pad], BF16)
            nc.sync.dma_start(out=mask_sb[:], in_=mask_in[:])
            maskT_sb = const.tile([128, E_pad], BF16)
            nc.sync.dma_start(out=maskT_sb[:], in_=maskT_in[:])

            # ---- wesum / A prep
            wesum_sb = const.tile([128, 2, 2 * H], F32)
            a1_sb = const.tile([128, 2, 2 * H], F32)
            a2_sb = const.tile([128, 2, 2 * H], F32)
            for ct in range(2):
                pw = ps_small.tile([128, 2 * H], F32, space="PSUM", tag="ps", name="pw")
                for lj, (wehd, hs) in enumerate(
                        [("we1hd", "hsel1"), ("we2hd", "hsel2")]):
                    for kh in range(2):
                        nc.tensor.matmul(
                            out=pw[:, lj * H:(lj + 1) * H],
                            lhsT=w_sb[wehd][:, kh, ct * 128:(ct + 1) * 128],
                            rhs=w_sb[hs][:, kh, :],
                            start=(kh == 0), stop=(kh == 1))
                nc.scalar.copy(out=wesum_sb[:, ct, :], in_=pw[:])
                for dst, wnhd, ab in [(a1_sb, "wn1hd", "ablk1"),
                                      (a2_sb, "wn2hd", "ablk2")]:
                    pa = ps_small.tile([128, 2 * H], F32, space="PSUM", tag="ps", name="pa")
                    for kh in range(2):
                        nc.tensor.matmul(
                            out=pa[:],
                            lhsT=w_sb[wnhd][:, kh, ct * 128:(ct + 1) * 128],
                            rhs=w_sb[ab][:, kh, :],
                            start=(kh == 0), stop=(kh == 1))
                    nc.scalar.copy(out=dst[:, ct, :], in_=pa[:])

            # ---- phase A: pe[e, (layer, h)] f32 for all edge slots
            pe_sb = sb.tile([128, n_chunks, 2 * H], F32)
            for q in range(Q):
                eft = efpool.tile([128, B_pad // 128, C], F32)
                nc.gpsimd.dma_gather(
                    out_ap=eft[:],
                    in_ap=ef_in[q * QROWS:(q + 1) * QROWS, :],
                    idxs_ap=ief_t[:, q * (B_pad // 16):(q + 1) * (B_pad // 16)],
                    num_idxs=B_pad, num_idxs_reg=B_pad, elem_size=C,
                    single_packet=False)
                for jc in range(B_pad // 128):
                    c = q * (B_pad // 128) + jc
                    eT = small.tile([128, 2, 128], F32, tag="eT")
                    for ch in range(2):
                        pt = ps_t.tile([128, 128], F32, space="PSUM", tag="pt", name="pt")
                        nc.tensor.transpose(
                            out=pt[:], in_=eft[:, jc, ch * 128:(ch + 1) * 128],
                            identity=ident[:])
                        nc.scalar.copy(out=eT[:, ch, :], in_=pt[:])
                    pp = ps_small.tile([128, 2 * H], F32, space="PSUM", tag="ps", name="pp")
                    for ch in range(2):
                        nc.tensor.matmul(
                            out=pp[:], lhsT=eT[:, ch, :],
                            rhs=wesum_sb[:, ch, :],
                            start=(ch == 0), stop=(ch == 1))
                    nc.scalar.copy(out=pe_sb[:, c, :], in_=pp[:])
            if debug:
                nc.sync.dma_start(out=dbg["dbg_pe"][:], in_=pe_sb[:])

            # ---- local table build (+ local a_tgt hi/res rhs) + AllGather
            def build_table(lhsT_sb, wncols, a_sb, tag):
                ag_in = dram.tile([TPC, ROW], BF16, tag=f"agin{tag}",
                                  name=f"agin{tag}")
                table = dram.tile([N, ROW], BF16, addr_space="Shared",
                                  tag=f"tbl{tag}", name=f"tbl{tag}")
                for t in range(4):
                    ph = ps_small.tile([128, C], F32, space="PSUM", tag="ps", name="ph")
                    pa = ps_small.tile([128, 2 * H], F32, space="PSUM", tag="ps", name="pa2")
                    for ch in range(2):
                        lhsT = lhsT_sb[:, ch, t * 128:(t + 1) * 128]
                        nc.tensor.matmul(out=ph[:], lhsT=lhsT,
                                         rhs=wncols[:, ch, :],
                                         start=(ch == 0), stop=(ch == 1))
                        nc.tensor.matmul(out=pa[:], lhsT=lhsT,
                                         rhs=a_sb[:, ch, :],
                                         start=(ch == 0), stop=(ch == 1))
                    sh = small.tile([128, C], BF16, tag="sh")
                    sa = small.tile([128, 2 * H], F32, tag="sa")
                    nc.scalar.copy(out=sh[:], in_=ph[:])
                    nc.scalar.copy(out=sa[:], in_=pa[:])
                    rows = slice(t * 32, (t + 1) * 32)
                    nc.sync.dma_start(
                        out=ag_in[rows, 0:B * C].rearrange(
                            "n (b o) -> n b o", b=B),
                        in_=sh[:])
                    nc.sync.dma_start(
                        out=ag_in[rows, AS_OFF:AS_OFF + 2 * B * H].bitcast(
                            F32).rearrange("n (b h) -> n b h", b=B),
                        in_=sa[:, 0:H])
                    nc.sync.dma_start(
                        out=ag_in[rows, AT_OFF:AT_OFF + 2 * B * H].bitcast(
                            F32).rearrange("n (b h) -> n b h", b=B),
                        in_=sa[:, H:2 * H])
                    nc.sync.dma_start(
                        out=ag_in[rows, AT_OFF + 2 * B * H:ROW].rearrange(
                            "n (b z) -> n b z", b=B),
                        in_=zpad[:])
                # local a_tgt[t, (b h)] via per-b matmuls, then hi/res split
                at_loc = small.tile([128, B * H], F32, tag="atl")
                for b in range(B):
                    pab = ps_small.tile([128, 2 * H], F32, space="PSUM",
                                        tag="ps", name="pab")
                    for ch in range(2):
                        lhsT_b = lhsT_sb[:, ch, :].rearrange(
                            "p (n b2) -> p b2 n", b2=B)[:, b, :]
                        nc.tensor.matmul(out=pab[:], lhsT=lhsT_b,
                                         rhs=a_sb[:, ch, :],
                                         start=(ch == 0), stop=(ch == 1))
                    nc.vector.tensor_copy(out=at_loc[:, b * H:(b + 1) * H],
                                          in_=pab[:, H:2 * H])
                at_rhs = small.tile([128, 2 * B * H], BF16, tag="atr")
                at_tmp = small.tile([128, B * H], F32, tag="att")
                nc.vector.tensor_copy(out=at_rhs[:, 0:B * H], in_=at_loc[:])
                nc.vector.tensor_copy(out=at_tmp[:], in_=at_rhs[:, 0:B * H])
                nc.vector.tensor_tensor(out=at_tmp[:], in0=at_loc[:],
                                        in1=at_tmp[:],
                                        op=mybir.AluOpType.subtract)
                nc.vector.tensor_copy(out=at_rhs[:, B * H:2 * B * H],
                                      in_=at_tmp[:])
                nc.gpsimd.collective_compute(
                    "AllGather", mybir.AluOpType.bypass,
                    replica_groups=[list(range(NC))],
                    ins=[ag_in.opt()], outs=[table.opt()])
                return table, at_rhs

            # ---- edge loop for one layer
            def edge_loop(table, at_rhs, layer):
                out_p = ps_out.tile([128, B * C], F32, space="PSUM", tag="out",
                                    name="out_p")
                den_p = ps_den.tile([128, B * H], F32, space="PSUM", tag="den",
                                    name="den_p")
                for s in range(n_super):
                    G = gpool.tile([128, 4, ROW], BF16, tag="G")
                    nc.gpsimd.dma_gather(
                        out_ap=G[:], in_ap=table[:],
                        idxs_ap=isrc_t[:, s * 32:(s + 1) * 32],
                        num_idxs=512, num_idxs_reg=512, elem_size=ROW,
                        single_packet=False)
                    for j in range(4):
                        c = s * 4 + j
                        pat = ps_small.tile([128, 2 * B * H], F32, space="PSUM",
                                            tag="ps", name="pat")
                        nc.tensor.matmul(
                            out=pat[:],
                            lhsT=maskT_sb[:, c * 128:(c + 1) * 128],
                            rhs=at_rhs[:], start=True, stop=True)
                        s_sb = small.tile([128, B * H], F32, tag="s")
                        t_sb = small.tile([128, B * H], F32, tag="t")
                        e_sb = small.tile([128, B * H], F32, tag="e")
                        e_bf = small.tile([128, B * H], BF16, tag="ebf")
                        nc.vector.tensor_tensor(
                            out=s_sb[:].rearrange("p (b h) -> p b h", b=B),
                            in0=G[:, j, AS_OFF:AS_OFF + 2 * B * H].bitcast(
                                F32).rearrange("p (b h) -> p b h", b=B),
                            in1=pe_sb[:, c:c + 1, layer * H:(layer + 1) * H]
                                .to_broadcast([128, B, H]),
                            op=mybir.AluOpType.add)
                        nc.vector.tensor_tensor(
                            out=s_sb[:], in0=s_sb[:], in1=pat[:, 0:B * H],
                            op=mybir.AluOpType.add)
                        nc.vector.tensor_tensor(
                            out=s_sb[:], in0=s_sb[:], in1=pat[:, B * H:2 * B * H],
                            op=mybir.AluOpType.add)
                        nc.scalar.mul(out=t_sb[:], in_=s_sb[:], mul=0.2)
                        nc.vector.tensor_tensor(
                            out=s_sb[:], in0=s_sb[:], in1=t_sb[:],
                            op=mybir.AluOpType.max)
                        nc.scalar.activation(
                            out=e_sb[:], in_=s_sb[:],
                            func=mybir.ActivationFunctionType.Exp)
                        nc.vector.tensor_copy(out=e_bf[:], in_=e_sb[:])
                        nc.vector.tensor_tensor(
                            out=G[:, j, 0:B * C].rearrange(
                                "p (x d) -> p x d", d=D),
                            in0=G[:, j, 0:B * C].rearrange(
                                "p (x d) -> p x d", d=D),
                            in1=e_bf[:].rearrange("p (x u) -> p x u", u=1)
                                .to_broadcast([128, B * H, D]),
                            op=mybir.AluOpType.mult)
                        mk = mask_sb[:, c * 128:(c + 1) * 128]
                        first, last = (c == 0), (c == n_chunks - 1)
                        nc.tensor.matmul(out=out_p[:, 0:512], lhsT=mk,
                                         rhs=G[:, j, 0:512],
                                         start=first, stop=last)
                        nc.tensor.matmul(out=out_p[:, 512:1024], lhsT=mk,
                                         rhs=G[:, j, 512:1024],
                                         start=first, stop=last)
                        nc.tensor.matmul(out=den_p[:], lhsT=mk, rhs=e_bf[:],
                                         start=first, stop=last)
                dsb = small.tile([128, B * H], F32, tag="d")
                nc.vector.tensor_scalar_add(dsb[:], den_p[:], 1e-16)
                rec = small.tile([128, B * H], F32, tag="r")
                nc.vector.reciprocal(rec[:], dsb[:])
                xo = sb.tile([128, B * C], F32, tag=f"xo{layer}",
                             name=f"xo{layer}")
                nc.vector.tensor_tensor(
                    out=xo[:].rearrange("p (x d) -> p x d", d=D),
                    in0=out_p[:].rearrange("p (x d) -> p x d", d=D),
                    in1=rec[:].rearrange("p (x u) -> p x u", u=1)
                        .to_broadcast([128, B * H, D]),
                    op=mybir.AluOpType.mult)
                return xo

            table1, at1 = build_table(xT_sb, w_sb["wn1cols"], a1_sb, 1)
            if stop_after == 'B':
                dummy = sb.tile([128, B * C], F32)
                nc.sync.dma_start(out=dummy[:],
                                  in_=table1[0:128, 0:2 * B * C].bitcast(F32))
                nc.sync.dma_start(out=y_out[:], in_=dummy[:])
            elif stop_after == 'C1':
                x1 = edge_loop(table1, at1, 0)
                nc.sync.dma_start(out=y_out[:], in_=x1[:])
            else:
                x1 = edge_loop(table1, at1, 0)
                if debug:
                    nc.sync.dma_start(out=dbg["dbg_x1"][:], in_=x1[:])
                    nc.sync.dma_start(out=dbg["dbg_tbl"][:], in_=table1[:])

                x1T = sb.tile([128, 2, NB_LOCAL], F32)
                for b in range(B):
                    for ch in range(2):
                        pt = ps_t.tile([128, 128], F32, space="PSUM", tag="pt",
                                       name="pt")
                        nc.tensor.transpose(
                            out=pt[:],
                            in_=x1[:, b * C + ch * 128: b * C + (ch + 1) * 128],
                            identity=ident[:])
                        nc.scalar.copy(
                            out=x1T[:, ch, :].rearrange(
                                "p (n b2) -> p n b2", b2=B)[:, :, b],
                            in_=pt[:])

                table2, at2 = build_table(x1T, w_sb["wn2cols"], a2_sb, 2)
                x2 = edge_loop(table2, at2, 1)
                nc.sync.dma_start(out=y_out[:], in_=x2[:])

    nc.compile()
    return nc


_CACHE: dict = {}


def _get_program(B_pad: int, debug: bool = False, stop_after: str = "full"):
    key = (B_pad, debug, stop_after)
    if key not in _CACHE:
        _CACHE[key] = _build(B_pad, debug, stop_after)
    return _CACHE[key]


def kernel(debug=False, trace=False, **inputs):
    in_maps, B_pad, E_pad, n_chunks = _prep(**inputs)
    nc = _get_program(B_pad, debug)
    res = run_bass_kernel_spmd(nc, in_maps, core_ids=list(range(NC)),
                               trace=trace)
    y = np.concatenate([res.results[k]["y"] for k in range(NC)], axis=0)
    out = y.reshape(N, B, C)
    if debug or trace:
        return out, res
    return out



# revision 19
# speedup vs baseline: 1.3987x; 1.3987x over previous
"""Trainium2 Bass kernel for a 2-layer GAT (nn_GAT_70909910057105).

Strategy (8 NeuronCores, SPMD):
  - Core k owns target nodes [128k, 128k+128). Edges bucketed by trg//128 on
    the host (layout-only preprocessing).
  - Edge features ef[src_e, trg_e] are host-gathered, transposed and staged
    bf16 as efT [128, 2, E_pad]; pe = efT.T @ wesum via direct matmuls.
  - Every core builds the FULL node table (all 1024 nodes) from replicated x
    with bf16 matmuls -> no layer-1 collective. Table rows in DRAM:
    [h bf16 x1024 (b-major) | a_src bf16 x16 (b,h) | pad] = 1152 bf16.
  - Per-edge source rows fetched with dma_gather (2304B rows), all gathers
    issued before the compute loop so Q7 descriptor-gen runs ahead.
  - Scores: PSUM-accumulated matmuls (maskT@a_tgt + I@pe + I@a_src); exp via
    leaky trick max(exp(s), exp(0.2 s)) on the Scalar engine, written as
    duplicated bf16 pairs so the DVE message-scaling multiply runs in 2x mode.
  - segment_sum via one-hot mask matmuls into PSUM (as before).
  - Layer 2: x1^T AllGather (bf16) -> full table2 build locally.
"""
import sys

for _p in ("/opt/trn_rl_repo", "/root/.axon_site/_ro/trn_rl_repo"):
    if _p not in sys.path:
        sys.path.insert(0, _p)

import numpy as np
import ml_dtypes
import concourse.bass as bass
import concourse.bacc as bacc
import concourse.tile as tile
from concourse import mybir
from concourse.bass_utils import run_bass_kernel_spmd
from concourse.masks import make_identity

F32 = mybir.dt.float32
BF16 = mybir.dt.bfloat16
I16 = mybir.dt.int16
NPBF = ml_dtypes.bfloat16

N, B, C, H, D = 1024, 4, 256, 4, 64
E = 32768
NC = 8
TPC = N // NC           # target nodes per core = 128
ROW = 1152              # bf16: 1024 h (b-major) | 16 a_src (b,h) | 112 pad
AS_OFF = 1024


# --------------------------------------------------------------------------
# host-side preprocessing (layout / gather only, no arithmetic)
# --------------------------------------------------------------------------

def _pack_idx(vals: np.ndarray) -> np.ndarray:
    n = vals.shape[0]
    assert n % 16 == 0
    blk = vals.astype(np.int16).reshape(n // 16, 16).T
    return np.ascontiguousarray(np.tile(blk, (8, 1)))


def _sb3(w):
    # [R, inner] f32/bf16 -> [128, R//128, inner] with partition = r % 128
    r, inner = w.shape
    return np.ascontiguousarray(
        w.reshape(r // 128, 128, inner).transpose(1, 0, 2)).astype(NPBF)


def _prep(x, edge_features, src_idx, trg_idx,
          Wn1, We1, a_src1, a_tgt1, a_edge1,
          Wn2, We2, a_src2, a_tgt2, a_edge2):
    src = np.asarray(src_idx).astype(np.int64)
    trg = np.asarray(trg_idx).astype(np.int64)
    x = np.asarray(x, dtype=np.float32)
    ef = np.asarray(edge_features, dtype=np.float32)

    buckets = [np.nonzero((trg // TPC) == k)[0] for k in range(NC)]
    bmax = max(len(b) for b in buckets)
    NC_E = (bmax + 127) // 128
    E_pad = NC_E * 128

    def ablk(a_s, a_t):
        m = np.zeros((C, 8), np.float32)
        for h in range(H):
            m[h * D:(h + 1) * D, h] = np.asarray(a_s)[h]
            m[h * D:(h + 1) * D, 4 + h] = np.asarray(a_t)[h]
        return _sb3(m)

    def hselr(a_e):
        m = np.zeros((C, 16), np.float32)
        for b in range(B):
            for h in range(H):
                m[h * D:(h + 1) * D, b * H + h] = np.float32(np.asarray(a_e)[h])
        return _sb3(m)

    def hi_res(w):
        w = np.asarray(w, np.float32)
        hi = w.astype(NPBF)
        res = (w - hi.astype(np.float32)).astype(NPBF)
        return hi, res

    # x transposed, b-major cols: xT[c%128, c//128, b, n] = x[n, b, c]
    xb = np.ascontiguousarray(x.transpose(2, 1, 0))          # [C, B, N]
    xT = np.ascontiguousarray(
        xb.reshape(2, 128, B, N).transpose(1, 0, 2, 3)).astype(NPBF)

    we1h, we1r = hi_res(We1)
    we2h, we2r = hi_res(We2)
    common = {
        "wa1": _sb3(np.asarray(Wn1, np.float32).T),
        "wa2": _sb3(np.asarray(Wn2, np.float32).T),
        "wn1hd": _sb3(np.asarray(Wn1, np.float32)),
        "wn2hd": _sb3(np.asarray(Wn2, np.float32)),
        "ablk1": ablk(a_src1, a_tgt1),
        "ablk2": ablk(a_src2, a_tgt2),
        "we1hd_hi": _sb3(we1h), "we1hd_res": _sb3(we1r),
        "we2hd_hi": _sb3(we2h), "we2hd_res": _sb3(we2r),
        "hselr1": hselr(a_edge1),
        "hselr2": hselr(a_edge2),
        "xT": xT,
    }

    in_maps = []
    for k in range(NC):
        eids = buckets[k]
        nk = len(eids)
        src_s = np.zeros(E_pad, np.int64)
        src_s[:nk] = src[eids]
        tloc = trg[eids] - k * TPC
        slots = np.arange(nk)
        mask = np.zeros((128, E_pad), np.float32)
        maskT = np.zeros((128, E_pad), np.float32)
        mask[slots % 128, (slots // 128) * 128 + tloc] = 1.0
        maskT[tloc, (slots // 128) * 128 + slots % 128] = 1.0
        efg = np.zeros((256, E_pad), np.float32)
        efg[:, :nk] = ef[src[eids], trg[eids]].T
        efs = np.ascontiguousarray(
            efg.reshape(2, 128, E_pad).transpose(1, 0, 2))
        efT_hi = efs.astype(NPBF)
        efT_res = (efs - efT_hi.astype(np.float32)).astype(NPBF)
        xTloc = np.ascontiguousarray(xT[:, :, :, k * TPC:(k + 1) * TPC])
        m = dict(common)
        m.update({
            "efT_hi": efT_hi,
            "efT_res": efT_res,
            "xTloc": xTloc,
            "isrc": _pack_idx(src_s),
            "mask": mask.astype(NPBF),
            "maskT": maskT.astype(NPBF),
        })
        in_maps.append(m)
    return in_maps, NC_E


# --------------------------------------------------------------------------
# device program
# --------------------------------------------------------------------------

def _build(NC_E: int, debug: bool = False):
    E_pad = NC_E * 128
    n_super = (NC_E + 3) // 4
    nc = bacc.Bacc("TRN2", target_bir_lowering=False, debug=False,
                   num_devices=NC)

    efT_hi_in = nc.dram_tensor("efT_hi", [128, 2, E_pad], BF16,
                               kind="ExternalInput")
    efT_res_in = nc.dram_tensor("efT_res", [128, 2, E_pad], BF16,
                                kind="ExternalInput")
    xT_in = nc.dram_tensor("xT", [128, 2, B, N], BF16, kind="ExternalInput")
    xTloc_in = nc.dram_tensor("xTloc", [128, 2, B, TPC], BF16,
                              kind="ExternalInput")
    isrc_in = nc.dram_tensor("isrc", [128, E_pad // 16], I16,
                             kind="ExternalInput")
    mask_in = nc.dram_tensor("mask", [128, E_pad], BF16, kind="ExternalInput")
    maskT_in = nc.dram_tensor("maskT", [128, E_pad], BF16,
                              kind="ExternalInput")
    w_in = {
        nm: nc.dram_tensor(nm, [128, 2, inner], BF16, kind="ExternalInput")
        for nm, inner in [
            ("wa1", 256), ("wa2", 256), ("wn1hd", C), ("wn2hd", C),
            ("ablk1", 8), ("ablk2", 8),
            ("we1hd_hi", C), ("we1hd_res", C),
            ("we2hd_hi", C), ("we2hd_res", C),
            ("hselr1", 16), ("hselr2", 16),
        ]
    }
    y_out = nc.dram_tensor("y", [128, B * C], F32, kind="ExternalOutput")
    dbg = {}
    if debug:
        for nm, shape, dt in [("dbg_x1", [128, B * C], BF16),
                              ("dbg_pe", [128, NC_E, 32], F32),
                              ("dbg_tbl", [N, ROW], BF16)]:
            dbg[nm] = nc.dram_tensor(nm, shape, dt, kind="ExternalOutput")

    from contextlib import ExitStack
    with tile.TileContext(nc) as tc:
        with ExitStack() as ctx:
            const = ctx.enter_context(tc.tile_pool(name="const", bufs=1))
            sb = ctx.enter_context(tc.tile_pool(name="sb", bufs=1))
            small = ctx.enter_context(tc.tile_pool(name="small", bufs=3))
            shpool = ctx.enter_context(tc.tile_pool(name="shpool", bufs=2))
            gpool = ctx.enter_context(
                tc.tile_pool(name="gpool", bufs=n_super))
            erep = ctx.enter_context(tc.tile_pool(name="erep", bufs=2))
            ps_small = ctx.enter_context(
                tc.tile_pool(name="ps_small", bufs=2, space="PSUM"))
            ps_pat = ps_small
            ps_t = ctx.enter_context(
                tc.tile_pool(name="ps_t", bufs=1, space="PSUM"))
            ps_out = ctx.enter_context(
                tc.tile_pool(name="ps_out", bufs=1, space="PSUM"))
            ps_den = ctx.enter_context(
                tc.tile_pool(name="ps_den", bufs=1, space="PSUM"))
            dram = ctx.enter_context(
                tc.tile_pool(name="dram", bufs=1, space="DRAM"))

            ident = const.tile([128, 128], BF16)
            make_identity(nc, ident[:])

            w_sb = {}
            for nm, t in w_in.items():
                if nm in ("wa1", "wa2"):
                    continue
                inner = t.shape[2]
                w_sb[nm] = const.tile([128, 2, inner], BF16, name=f"w_{nm}",
                                      tag=f"w_{nm}")
                nc.sync.dma_start(out=w_sb[nm][:], in_=t[:])
            xT_sb = const.tile([128, 2, B, N], BF16)
            nc.sync.dma_start(out=xT_sb[:], in_=xT_in[:])
            xTloc_sb = const.tile([128, 2, B, TPC], BF16)
            nc.sync.dma_start(out=xTloc_sb[:], in_=xTloc_in[:])
            efT_hi_sb = const.tile([128, 2, E_pad], BF16, name="efthi")
            nc.sync.dma_start(out=efT_hi_sb[:], in_=efT_hi_in[:])
            efT_res_sb = const.tile([128, 2, E_pad], BF16, name="eftres")
            nc.sync.dma_start(out=efT_res_sb[:], in_=efT_res_in[:])
            isrc_t = const.tile([128, E_pad // 16], I16)
            nc.sync.dma_start(out=isrc_t[:], in_=isrc_in[:])
            mask_sb = const.tile([128, E_pad], BF16)
            nc.sync.dma_start(out=mask_sb[:], in_=mask_in[:])
            maskT_sb = const.tile([128, E_pad], BF16)
            nc.sync.dma_start(out=maskT_sb[:], in_=maskT_in[:])

            # ---- wesum[c_in, (l, b, h)] as bf16 hi/res pair
            wesum_hi = const.tile([128, 2, 32], BF16, name="wesh")
            wesum_res = const.tile([128, 2, 32], BF16, name="wesr")
            for ct in range(2):
                pw = ps_small.tile([128, 32], F32, space="PSUM", tag="ps",
                                   name="pw")
                for lj, (weh, wer, hs) in enumerate(
                        [("we1hd_hi", "we1hd_res", "hselr1"),
                         ("we2hd_hi", "we2hd_res", "hselr2")]):
                    for mi, wnm in enumerate([weh, wer]):
                        for kh in range(2):
                            nc.tensor.matmul(
                                out=pw[:, lj * 16:(lj + 1) * 16],
                                lhsT=w_sb[wnm][:, kh,
                                               ct * 128:(ct + 1) * 128],
                                rhs=w_sb[hs][:, kh, :],
                                start=(mi == 0 and kh == 0),
                                stop=(mi == 1 and kh == 1))
                nc.scalar.copy(out=wesum_hi[:, ct, :], in_=pw[:])
                wtmp = small.tile([128, 32], F32, tag="wtmp")
                nc.vector.tensor_tensor(out=wtmp[:], in0=pw[:],
                                        in1=wesum_hi[:, ct, :],
                                        op=mybir.AluOpType.subtract)
                nc.vector.tensor_copy(out=wesum_res[:, ct, :], in_=wtmp[:])

            # ---- wa{l} = [Wn.T cols | projected a-cols (device-computed)]
            wa_sb = {}
            for l, (wanm, wnhd, ab) in enumerate(
                    [("wa1", "wn1hd", "ablk1"), ("wa2", "wn2hd", "ablk2")]):
                wt = const.tile([128, 2, 264], BF16, name=f"wt{l}",
                                tag=f"wt{l}")
                nc.sync.dma_start(out=wt[:, :, 0:256], in_=w_in[wanm][:])
                for ct in range(2):
                    pac = ps_small.tile([128, 8], F32, space="PSUM",
                                        tag="ps", name="pac")
                    for kh in range(2):
                        nc.tensor.matmul(
                            out=pac[:],
                            lhsT=w_sb[wnhd][:, kh, ct * 128:(ct + 1) * 128],
                            rhs=w_sb[ab][:, kh, :],
                            start=(kh == 0), stop=(kh == 1))
                    nc.scalar.copy(out=wt[:, ct, 256:264], in_=pac[:])
                wa_sb[wanm] = wt

            # ---- full-table build: rows [h (b,256) | a_src (b,h)]
            def build_table(lhsT_fn, wname, tag):
                table = dram.tile([N, ROW], BF16, tag=f"tbl{tag}",
                                  name=f"tbl{tag}")
                for g in range(8):
                    sh = shpool.tile([128, B, 264], BF16, tag="sh")
                    for b in range(B):
                        ph = ps_small.tile([128, 264], F32, space="PSUM",
                                           tag="ps", name="ph")
                        for ch in range(2):
                            nc.tensor.matmul(
                                out=ph[:], lhsT=lhsT_fn(ch, b, g),
                                rhs=wa_sb[wname][:, ch, :],
                                start=(ch == 0), stop=(ch == 1))
                        nc.scalar.copy(out=sh[:, b, :], in_=ph[:])
                    rows = slice(g * 128, (g + 1) * 128)
                    nc.sync.dma_start(
                        out=table[rows, 0:B * C].rearrange(
                            "n (b o) -> n b o", b=B),
                        in_=sh[:, :, 0:256])
                    nc.sync.dma_start(
                        out=table[rows, AS_OFF:AS_OFF + B * H].rearrange(
                            "n (b h) -> n b h", b=B),
                        in_=sh[:, :, 256:260])
                return table

            # ---- local a_tgt [t, (b, h)] bf16
            def at_local(loc_fn, wname, tag):
                at = sb.tile([128, B * H], BF16, tag=f"at{tag}",
                             name=f"at{tag}")
                for b in range(B):
                    pa = ps_small.tile([128, 8], F32, space="PSUM", tag="ps",
                                       name="pa")
                    for ch in range(2):
                        nc.tensor.matmul(
                            out=pa[:], lhsT=loc_fn(ch, b),
                            rhs=wa_sb[wname][:, ch, 256:264],
                            start=(ch == 0), stop=(ch == 1))
                    nc.scalar.copy(out=at[:, b * H:(b + 1) * H],
                                   in_=pa[:, 4:8])
                return at

            table1 = build_table(
                lambda ch, b, g: xT_sb[:, ch, b, g * 128:(g + 1) * 128],
                "wa1", 1)
            at1 = at_local(lambda ch, b: xTloc_sb[:, ch, b, :], "wa1", 1)

            # ---- pe[e, (l, b, h)] f32 for all edge slots
            # hi/res cross terms keep score error ~1e-4 with bf16 operands
            pe_sb = sb.tile([128, NC_E, 32], F32)
            for c in range(NC_E):
                pp = ps_small.tile([128, 32], F32, space="PSUM", tag="ps",
                                   name="pp")
                terms = [(efT_hi_sb, wesum_hi), (efT_hi_sb, wesum_res),
                         (efT_res_sb, wesum_hi)]
                for ti, (eft, wes) in enumerate(terms):
                    for ch in range(2):
                        nc.tensor.matmul(
                            out=pp[:],
                            lhsT=eft[:, ch, c * 128:(c + 1) * 128],
                            rhs=wes[:, ch, :],
                            start=(ti == 0 and ch == 0),
                            stop=(ti == 2 and ch == 1))
                nc.scalar.copy(out=pe_sb[:, c, :], in_=pp[:])
            if debug:
                nc.sync.dma_start(out=dbg["dbg_pe"][:], in_=pe_sb[:])

            # ---- edge loop for one layer
            def edge_loop(table, at, layer, out_dt):
                out_p = ps_out.tile([128, B * C], F32, space="PSUM",
                                    tag="out", name="out_p")
                den_p = ps_den.tile([128, 32], F32, space="PSUM", tag="den",
                                    name="den_p")
                Gs = []
                for s in range(n_super):
                    r = min(4, NC_E - s * 4)
                    G = gpool.tile([128, 4, ROW], BF16, tag="G")
                    nc.gpsimd.dma_gather(
                        out_ap=G[:, 0:r, :], in_ap=table[:],
                        idxs_ap=isrc_t[:, s * 32:s * 32 + 8 * r],
                        num_idxs=128 * r, num_idxs_reg=128 * r,
                        elem_size=ROW, single_packet=False)
                    Gs.append(G)
                for s in range(n_super):
                    r = min(4, NC_E - s * 4)
                    G = Gs[s]
                    pat = ps_pat.tile([128, 4, 16], F32, space="PSUM",
                                      tag="pat", name="pat")
                    for j in range(r):
                        c = s * 4 + j
                        nc.tensor.matmul(
                            out=pat[:, j, :],
                            lhsT=maskT_sb[:, c * 128:(c + 1) * 128],
                            rhs=at[:], start=True, stop=False)
                        nc.tensor.matmul(
                            out=pat[:, j, :], lhsT=ident[:],
                            rhs=G[:, j, AS_OFF:AS_OFF + 16],
                            start=False, stop=True)
                    s_sb = erep.tile([128, 4, 16], F32, tag="s")
                    nc.vector.tensor_tensor(
                        out=s_sb[:, 0:r, :], in0=pat[:, 0:r, :],
                        in1=pe_sb[:, s * 4:s * 4 + r,
                                  layer * 16:(layer + 1) * 16],
                        op=mybir.AluOpType.add)
                    e_rep = erep.tile([128, 4, 16, 2], BF16, tag="e")
                    t_rep = erep.tile([128, 4, 16, 2], BF16, tag="t")
                    for half in range(2):
                        nc.scalar.activation(
                            out=e_rep[:, 0:r, :, half], in_=s_sb[:, 0:r, :],
                            func=mybir.ActivationFunctionType.Exp, scale=1.0)
                        nc.scalar.activation(
                            out=t_rep[:, 0:r, :, half], in_=s_sb[:, 0:r, :],
                            func=mybir.ActivationFunctionType.Exp, scale=0.2)
                    nc.vector.tensor_tensor(
                        out=e_rep[:, 0:r, :, :], in0=e_rep[:, 0:r, :, :],
                        in1=t_rep[:, 0:r, :, :], op=mybir.AluOpType.max)
                    for j in range(r):
                        c = s * 4 + j
                        nc.vector.tensor_tensor(
                            out=G[:, j, 0:B * C].rearrange(
                                "p (x pr two) -> p x pr two", pr=32, two=2),
                            in0=G[:, j, 0:B * C].rearrange(
                                "p (x pr two) -> p x pr two", pr=32, two=2),
                            in1=e_rep[:, j, :, :].rearrange(
                                "p x (u two) -> p x u two", u=1)
                                .to_broadcast([128, B * H, 32, 2]),
                            op=mybir.AluOpType.mult)
                        mk = mask_sb[:, c * 128:(c + 1) * 128]
                        first, last = (c == 0), (c == NC_E - 1)
                        nc.tensor.matmul(out=out_p[:, 0:512], lhsT=mk,
                                         rhs=G[:, j, 0:512],
                                         start=first, stop=last)
                        nc.tensor.matmul(out=out_p[:, 512:1024], lhsT=mk,
                                         rhs=G[:, j, 512:1024],
                                         start=first, stop=last)
                        nc.tensor.matmul(
                            out=den_p[:], lhsT=mk,
                            rhs=e_rep[:, j, :, :].rearrange(
                                "p x two -> p (x two)"),
                            start=first, stop=last)
                dsb = small.tile([128, B * H], F32, tag="d")
                nc.vector.tensor_scalar_add(
                    dsb[:],
                    den_p[:].rearrange("p (x two) -> p x two", two=2)[:, :, 0],
                    1e-16)
                rec = small.tile([128, B * H], F32, tag="r")
                nc.vector.reciprocal(rec[:], dsb[:])
                xo = sb.tile([128, B * C], out_dt, tag=f"xo{layer}",
                             name=f"xo{layer}")
                nc.vector.tensor_tensor(
                    out=xo[:].rearrange("p (x d) -> p x d", d=D),
                    in0=out_p[:].rearrange("p (x d) -> p x d", d=D),
                    in1=rec[:].rearrange("p (x u) -> p x u", u=1)
                        .to_broadcast([128, B * H, D]),
                    op=mybir.AluOpType.mult)
                return xo

            x1 = edge_loop(table1, at1, 0, BF16)
            if debug:
                nc.sync.dma_start(out=dbg["dbg_x1"][:], in_=x1[:])
                nc.sync.dma_start(out=dbg["dbg_tbl"][:], in_=table1[:])

            # ---- x1^T (local), AllGather, full x1^T
            x1T_loc = sb.tile([128, 2, B * TPC], BF16)
            for b in range(B):
                for ch in range(2):
                    pt = ps_t.tile([128, 128], BF16, space="PSUM", tag="pt",
                                   name="pt")
                    nc.tensor.transpose(
                        out=pt[:],
                        in_=x1[:, b * C + ch * 128: b * C + (ch + 1) * 128],
                        identity=ident[:])
                    nc.scalar.copy(
                        out=x1T_loc[:, ch, b * 128:(b + 1) * 128], in_=pt[:])
            ag_in = dram.tile([128, 2 * B * TPC], BF16, tag="agin",
                              name="agin")
            ag_out = dram.tile([N, 2 * B * TPC], BF16, addr_space="Shared",
                               tag="agout", name="agout")
            nc.sync.dma_start(
                out=ag_in[:].rearrange("p (ch n) -> p ch n", ch=2),
                in_=x1T_loc[:])
            nc.gpsimd.collective_compute(
                "AllGather", mybir.AluOpType.bypass,
                replica_groups=[list(range(NC))],
                ins=[ag_in.opt()], outs=[ag_out.opt()])
            x1T_full = sb.tile([128, 2, B, N], BF16)
            for k in range(NC):
                nc.sync.dma_start(
                    out=x1T_full[:, :, :, k * 128:(k + 1) * 128],
                    in_=ag_out[k * 128:(k + 1) * 128, :].rearrange(
                        "p (ch b t) -> p ch b t", ch=2, b=B))

            table2 = build_table(
                lambda ch, b, g: x1T_full[:, ch, b, g * 128:(g + 1) * 128],
                "wa2", 2)
            at2 = at_local(
                lambda ch, b: x1T_loc[:, ch, b * 128:(b + 1) * 128], "wa2", 2)

            x2 = edge_loop(table2, at2, 1, F32)
            nc.sync.dma_start(out=y_out[:], in_=x2[:])

    nc.compile()
    return nc


_CACHE: dict = {}


def _get_program(NC_E: int, debug: bool = False):
    key = (NC_E, debug)
    if key not in _CACHE:
        _CACHE[key] = _build(NC_E, debug)
    return _CACHE[key]


def kernel(debug=False, trace=False, **inputs):
    in_maps, NC_E = _prep(**inputs)
    nc = _get_program(NC_E, debug)
    res = run_bass_kernel_spmd(nc, in_maps, core_ids=list(range(NC)),
                               trace=trace)
    y = np.concatenate([res.results[k]["y"] for k in range(NC)], axis=0)
    out = y.reshape(N, B, C)
    if debug or trace:
        return out, res
    return out


# revision 26
# speedup vs baseline: 1.4699x; 1.0510x over previous
"""Trainium2 Bass kernel for a 2-layer GAT (nn_GAT_70909910057105).

Strategy (8 NeuronCores, SPMD):
  - Core k owns target nodes [128k, 128k+128). Edges bucketed by trg//128 on
    the host (layout-only preprocessing).
  - Edge features ef[src_e, trg_e] are host-gathered, transposed and staged
    bf16 as efT [128, 2, E_pad]; pe = efT.T @ wesum via direct matmuls.
  - Every core builds the FULL node table (all 1024 nodes) from replicated x
    with bf16 matmuls -> no layer-1 collective. Table rows in DRAM:
    [h bf16 x1024 (b-major) | a_src bf16 x16 (b,h) | pad] = 1152 bf16.
  - Per-edge source rows fetched with dma_gather (2304B rows), all gathers
    issued before the compute loop so Q7 descriptor-gen runs ahead.
  - Scores: PSUM-accumulated matmuls (maskT@a_tgt + I@pe + I@a_src); exp via
    leaky trick max(exp(s), exp(0.2 s)) on the Scalar engine, written as
    duplicated bf16 pairs so the DVE message-scaling multiply runs in 2x mode.
  - segment_sum via one-hot mask matmuls into PSUM (as before).
  - Layer 2: x1^T AllGather (bf16) -> full table2 build locally.
"""
import sys

for _p in ("/opt/trn_rl_repo", "/root/.axon_site/_ro/trn_rl_repo"):
    if _p not in sys.path:
        sys.path.insert(0, _p)

import numpy as np
import ml_dtypes
import concourse.bass as bass
import concourse.bacc as bacc
import concourse.tile as tile
from concourse import mybir
from concourse.bass_utils import run_bass_kernel_spmd
from concourse.masks import make_identity

F32 = mybir.dt.float32
BF16 = mybir.dt.bfloat16
I16 = mybir.dt.int16
NPBF = ml_dtypes.bfloat16

N, B, C, H, D = 1024, 4, 256, 4, 64
E = 32768
NC = 8
TPC = N // NC           # target nodes per core = 128
ROW = 1152              # bf16: 1024 h (b-major) | 16 a_src (b,h) | 112 pad
AS_OFF = 1024


# --------------------------------------------------------------------------
# host-side preprocessing (layout / gather only, no arithmetic)
# --------------------------------------------------------------------------

def _pack_idx(vals: np.ndarray) -> np.ndarray:
    n = vals.shape[0]
    assert n % 16 == 0
    blk = vals.astype(np.int16).reshape(n // 16, 16).T
    return np.ascontiguousarray(np.tile(blk, (8, 1)))


def _sb3(w):
    # [R, inner] f32/bf16 -> [128, R//128, inner] with partition = r % 128
    r, inner = w.shape
    return np.ascontiguousarray(
        w.reshape(r // 128, 128, inner).transpose(1, 0, 2)).astype(NPBF)


def _prep(x, edge_features, src_idx, trg_idx,
          Wn1, We1, a_src1, a_tgt1, a_edge1,
          Wn2, We2, a_src2, a_tgt2, a_edge2):
    src = np.asarray(src_idx).astype(np.int64)
    trg = np.asarray(trg_idx).astype(np.int64)
    x = np.asarray(x, dtype=np.float32)
    ef = np.asarray(edge_features, dtype=np.float32)

    buckets = [np.nonzero((trg // TPC) == k)[0] for k in range(NC)]
    bmax = max(len(b) for b in buckets)
    NC_E = (bmax + 127) // 128
    E_pad = NC_E * 128

    def ablk(a_s, a_t):
        m = np.zeros((C, 8), np.float32)
        for h in range(H):
            m[h * D:(h + 1) * D, h] = np.asarray(a_s)[h]
            m[h * D:(h + 1) * D, 4 + h] = np.asarray(a_t)[h]
        return _sb3(m)



    # x transposed, b-major cols: xT[c%128, c//128, b, n] = x[n, b, c]
    xb = np.ascontiguousarray(x.transpose(2, 1, 0))          # [C, B, N]
    xT = np.ascontiguousarray(
        xb.reshape(2, 128, B, N).transpose(1, 0, 2, 3)).astype(NPBF)

    def _sb3f(w):
        r, inner = w.shape
        return np.ascontiguousarray(
            np.asarray(w, np.float32).reshape(r // 128, 128, inner)
            .transpose(1, 0, 2))

    common = {
        "wa1": _sb3(np.asarray(Wn1, np.float32).T),
        "wa2": _sb3(np.asarray(Wn2, np.float32).T),
        "wn1hd": _sb3(np.asarray(Wn1, np.float32)),
        "wn2hd": _sb3(np.asarray(Wn2, np.float32)),
        "ablk1": ablk(a_src1, a_tgt1),
        "ablk2": ablk(a_src2, a_tgt2),
        "we1hd": _sb3f(np.asarray(We1, np.float32)),
        "we2hd": _sb3f(np.asarray(We2, np.float32)),
        "hselr1": _sb3f(np.zeros((C, 16), np.float32) + 0.0),
        "hselr2": _sb3f(np.zeros((C, 16), np.float32) + 0.0),
        "xT": xT,
    }
    hs1 = np.zeros((C, 16), np.float32)
    hs2 = np.zeros((C, 16), np.float32)
    for b in range(B):
        for h in range(H):
            hs1[h * D:(h + 1) * D, b * H + h] = np.float32(
                np.asarray(a_edge1)[h])
            hs2[h * D:(h + 1) * D, b * H + h] = np.float32(
                np.asarray(a_edge2)[h])
    common["hselr1"] = _sb3f(hs1)
    common["hselr2"] = _sb3f(hs2)

    in_maps = []
    for k in range(NC):
        eids = buckets[k]
        nk = len(eids)
        src_s = np.zeros(E_pad, np.int64)
        src_s[:nk] = src[eids]
        tloc = trg[eids] - k * TPC
        slots = np.arange(nk)
        mask = np.zeros((128, E_pad), np.float32)
        maskT = np.zeros((128, E_pad), np.float32)
        mask[slots % 128, (slots // 128) * 128 + tloc] = 1.0
        maskT[tloc, (slots // 128) * 128 + slots % 128] = 1.0
        efg = np.zeros((256, E_pad), np.float32)
        efg[:, :nk] = ef[src[eids], trg[eids]].T
        efT = np.ascontiguousarray(
            efg.reshape(2, 128, E_pad).transpose(1, 0, 2))
        xTloc = np.ascontiguousarray(xT[:, :, :, k * TPC:(k + 1) * TPC])
        m = dict(common)
        m.update({
            "efT": efT,
            "xTloc": xTloc,
            "isrc": _pack_idx(src_s),
            "mask": mask.astype(NPBF),
            "maskT": maskT.astype(NPBF),
        })
        in_maps.append(m)
    return in_maps, NC_E


# --------------------------------------------------------------------------
# device program
# --------------------------------------------------------------------------

def _build(NC_E: int, debug: bool = False):
    E_pad = NC_E * 128
    n_super = (NC_E + 3) // 4
    nc = bacc.Bacc("TRN2", target_bir_lowering=False, debug=False,
                   num_devices=NC)

    efT_in = nc.dram_tensor("efT", [128, 2, E_pad], F32,
                            kind="ExternalInput")
    xT_in = nc.dram_tensor("xT", [128, 2, B, N], BF16, kind="ExternalInput")
    xTloc_in = nc.dram_tensor("xTloc", [128, 2, B, TPC], BF16,
                              kind="ExternalInput")
    isrc_in = nc.dram_tensor("isrc", [128, E_pad // 16], I16,
                             kind="ExternalInput")
    mask_in = nc.dram_tensor("mask", [128, E_pad], BF16, kind="ExternalInput")
    maskT_in = nc.dram_tensor("maskT", [128, E_pad], BF16,
                              kind="ExternalInput")
    w_in = {
        nm: nc.dram_tensor(nm, [128, 2, inner], BF16, kind="ExternalInput")
        for nm, inner in [
            ("wa1", 256), ("wa2", 256), ("wn1hd", C), ("wn2hd", C),
            ("ablk1", 8), ("ablk2", 8),
        ]
    }
    wf_in = {
        nm: nc.dram_tensor(nm, [128, 2, inner], F32, kind="ExternalInput")
        for nm, inner in [
            ("we1hd", C), ("we2hd", C), ("hselr1", 16), ("hselr2", 16),
        ]
    }
    y_out = nc.dram_tensor("y", [128, B * C], F32, kind="ExternalOutput")
    dbg = {}
    if debug:
        for nm, shape, dt in [("dbg_x1", [128, B * C], BF16),
                              ("dbg_pe", [128, NC_E, 32], F32),
                              ("dbg_tbl", [N, ROW], BF16)]:
            dbg[nm] = nc.dram_tensor(nm, shape, dt, kind="ExternalOutput")

    from contextlib import ExitStack
    with tile.TileContext(nc) as tc:
        with ExitStack() as ctx:
            const = ctx.enter_context(tc.tile_pool(name="const", bufs=1))
            sb = ctx.enter_context(tc.tile_pool(name="sb", bufs=1))
            small = ctx.enter_context(tc.tile_pool(name="small", bufs=3))
            shpool = ctx.enter_context(tc.tile_pool(name="shpool", bufs=2))
            gpool = ctx.enter_context(
                tc.tile_pool(name="gpool", bufs=n_super))
            erep = ctx.enter_context(tc.tile_pool(name="erep", bufs=2))
            ps_small = ctx.enter_context(
                tc.tile_pool(name="ps_small", bufs=2, space="PSUM"))
            ps_pat = ps_small
            ps_t = ctx.enter_context(
                tc.tile_pool(name="ps_t", bufs=1, space="PSUM"))
            ps_out = ctx.enter_context(
                tc.tile_pool(name="ps_out", bufs=1, space="PSUM"))
            ps_den = ctx.enter_context(
                tc.tile_pool(name="ps_den", bufs=1, space="PSUM"))
            dram = ctx.enter_context(
                tc.tile_pool(name="dram", bufs=1, space="DRAM"))

            ident = const.tile([128, 128], BF16)
            make_identity(nc, ident[:])

            # xT first: the table-1 build (the critical-path head) needs it
            xT_sb = const.tile([128, 2, B, N], BF16)
            nc.sync.dma_start(out=xT_sb[:], in_=xT_in[:])
            w_sb = {}
            for nm, t in w_in.items():
                if nm in ("wa1", "wa2"):
                    continue
                inner = t.shape[2]
                w_sb[nm] = const.tile([128, 2, inner], BF16, name=f"w_{nm}",
                                      tag=f"w_{nm}")
                nc.sync.dma_start(out=w_sb[nm][:], in_=t[:])
            for nm, t in wf_in.items():
                inner = t.shape[2]
                w_sb[nm] = const.tile([128, 2, inner], F32, name=f"w_{nm}",
                                      tag=f"w_{nm}")
                nc.sync.dma_start(out=w_sb[nm][:], in_=t[:])
            xTloc_sb = const.tile([128, 2, B, TPC], BF16)
            nc.sync.dma_start(out=xTloc_sb[:], in_=xTloc_in[:])
            isrc_t = const.tile([128, E_pad // 16], I16)
            nc.sync.dma_start(out=isrc_t[:], in_=isrc_in[:])
            mask_sb = const.tile([128, E_pad], BF16)
            nc.sync.dma_start(out=mask_sb[:], in_=mask_in[:])
            maskT_sb = const.tile([128, E_pad], BF16)
            nc.sync.dma_start(out=maskT_sb[:], in_=maskT_in[:])
            efT_sb = const.tile([128, 2, E_pad], F32, name="eft")
            nc.sync.dma_start(out=efT_sb[:], in_=efT_in[:])

            # ---- wesum[c_in, (l, b, h)] f32
            wesum_sb = const.tile([128, 2, 32], F32, name="wes")
            for ct in range(2):
                pw = ps_small.tile([128, 32], F32, space="PSUM", tag="ps",
                                   name="pw")
                for lj, (wehd, hs) in enumerate(
                        [("we1hd", "hselr1"), ("we2hd", "hselr2")]):
                    for kh in range(2):
                        nc.tensor.matmul(
                            out=pw[:, lj * 16:(lj + 1) * 16],
                            lhsT=w_sb[wehd][:, kh, ct * 128:(ct + 1) * 128],
                            rhs=w_sb[hs][:, kh, :],
                            start=(kh == 0), stop=(kh == 1))
                nc.scalar.copy(out=wesum_sb[:, ct, :], in_=pw[:])

            # ---- wa{l} = [Wn.T cols | projected a-cols (device-computed)]
            wa_sb = {}
            for l, (wanm, wnhd, ab) in enumerate(
                    [("wa1", "wn1hd", "ablk1"), ("wa2", "wn2hd", "ablk2")]):
                wt = const.tile([128, 2, 264], BF16, name=f"wt{l}",
                                tag=f"wt{l}")
                nc.sync.dma_start(out=wt[:, :, 0:256], in_=w_in[wanm][:])
                for ct in range(2):
                    pac = ps_small.tile([128, 8], F32, space="PSUM",
                                        tag="ps", name="pac")
                    for kh in range(2):
                        nc.tensor.matmul(
                            out=pac[:],
                            lhsT=w_sb[wnhd][:, kh, ct * 128:(ct + 1) * 128],
                            rhs=w_sb[ab][:, kh, :],
                            start=(kh == 0), stop=(kh == 1))
                    nc.scalar.copy(out=wt[:, ct, 256:264], in_=pac[:])
                wa_sb[wanm] = wt

            # ---- full-table build: rows [h (b,256) | a_src (b,h)]
            def build_table(lhsT_fn, wname, tag):
                table = dram.tile([N, ROW], BF16, tag=f"tbl{tag}",
                                  name=f"tbl{tag}")
                for g in range(8):
                    sh = shpool.tile([128, B, 264], BF16, tag="sh")
                    for b in range(B):
                        ph = ps_small.tile([128, 264], F32, space="PSUM",
                                           tag="ps", name="ph")
                        for ch in range(2):
                            nc.tensor.matmul(
                                out=ph[:], lhsT=lhsT_fn(ch, b, g),
                                rhs=wa_sb[wname][:, ch, :],
                                start=(ch == 0), stop=(ch == 1))
                        nc.scalar.copy(out=sh[:, b, :], in_=ph[:])
                    rows = slice(g * 128, (g + 1) * 128)
                    nc.sync.dma_start(
                        out=table[rows, 0:B * C].rearrange(
                            "n (b o) -> n b o", b=B),
                        in_=sh[:, :, 0:256])
                    nc.sync.dma_start(
                        out=table[rows, AS_OFF:AS_OFF + B * H].rearrange(
                            "n (b h) -> n b h", b=B),
                        in_=sh[:, :, 256:260])
                return table

            # ---- local a_tgt [t, (b, h)] bf16
            def at_local(loc_fn, wname, tag):
                at = sb.tile([128, B * H], BF16, tag=f"at{tag}",
                             name=f"at{tag}")
                for b in range(B):
                    pa = ps_small.tile([128, 8], F32, space="PSUM", tag="ps",
                                       name="pa")
                    for ch in range(2):
                        nc.tensor.matmul(
                            out=pa[:], lhsT=loc_fn(ch, b),
                            rhs=wa_sb[wname][:, ch, 256:264],
                            start=(ch == 0), stop=(ch == 1))
                    nc.scalar.copy(out=at[:, b * H:(b + 1) * H],
                                   in_=pa[:, 4:8])
                return at

            table1 = build_table(
                lambda ch, b, g: xT_sb[:, ch, b, g * 128:(g + 1) * 128],
                "wa1", 1)
            at1 = at_local(lambda ch, b: xTloc_sb[:, ch, b, :], "wa1", 1)

            # ---- pe[e, (l, b, h)] f32 for all edge slots
            pe_sb = sb.tile([128, NC_E, 32], F32)
            for c in range(NC_E):
                pp = ps_small.tile([128, 32], F32, space="PSUM", tag="ps",
                                   name="pp")
                for ch in range(2):
                    nc.tensor.matmul(
                        out=pp[:],
                        lhsT=efT_sb[:, ch, c * 128:(c + 1) * 128],
                        rhs=wesum_sb[:, ch, :],
                        start=(ch == 0), stop=(ch == 1))
                nc.scalar.copy(out=pe_sb[:, c, :], in_=pp[:])
            if debug:
                nc.sync.dma_start(out=dbg["dbg_pe"][:], in_=pe_sb[:])

            # ---- edge loop for one layer
            def edge_loop(table, at, layer, out_dt):
                out_p = ps_out.tile([128, B * C], F32, space="PSUM",
                                    tag="out", name="out_p")
                den_p = ps_den.tile([128, 32], F32, space="PSUM", tag="den",
                                    name="den_p")
                Gs = []
                for s in range(n_super):
                    r = min(4, NC_E - s * 4)
                    G = gpool.tile([128, 4, ROW], BF16, tag="G")
                    nc.gpsimd.dma_gather(
                        out_ap=G[:, 0:r, :], in_ap=table[:],
                        idxs_ap=isrc_t[:, s * 32:s * 32 + 8 * r],
                        num_idxs=128 * r, num_idxs_reg=128 * r,
                        elem_size=ROW, single_packet=False)
                    Gs.append(G)
                for s in range(n_super):
                    r = min(4, NC_E - s * 4)
                    G = Gs[s]
                    pat = ps_pat.tile([128, 4, 16], F32, space="PSUM",
                                      tag="pat", name="pat")
                    for j in range(r):
                        c = s * 4 + j
                        nc.tensor.matmul(
                            out=pat[:, j, :],
                            lhsT=maskT_sb[:, c * 128:(c + 1) * 128],
                            rhs=at[:], start=True, stop=True)
                    s_sb = erep.tile([128, 4, 16], F32, tag="s")
                    nc.vector.tensor_tensor(
                        out=s_sb[:, 0:r, :], in0=pat[:, 0:r, :],
                        in1=pe_sb[:, s * 4:s * 4 + r,
                                  layer * 16:(layer + 1) * 16],
                        op=mybir.AluOpType.add)
                    nc.vector.tensor_tensor(
                        out=s_sb[:, 0:r, :], in0=s_sb[:, 0:r, :],
                        in1=G[:, 0:r, AS_OFF:AS_OFF + 16],
                        op=mybir.AluOpType.add)
                    e_rep = erep.tile([128, 4, 16, 2], BF16, tag="e")
                    t_rep = erep.tile([128, 4, 16, 2], BF16, tag="t")
                    for half in range(2):
                        nc.scalar.activation(
                            out=e_rep[:, 0:r, :, half], in_=s_sb[:, 0:r, :],
                            func=mybir.ActivationFunctionType.Exp, scale=1.0)
                        nc.scalar.activation(
                            out=t_rep[:, 0:r, :, half], in_=s_sb[:, 0:r, :],
                            func=mybir.ActivationFunctionType.Exp, scale=0.2)
                    nc.vector.tensor_tensor(
                        out=e_rep[:, 0:r, :, :], in0=e_rep[:, 0:r, :, :],
                        in1=t_rep[:, 0:r, :, :], op=mybir.AluOpType.max)
                    for j in range(r):
                        c = s * 4 + j
                        nc.vector.tensor_tensor(
                            out=G[:, j, 0:B * C].rearrange(
                                "p (x pr two) -> p x pr two", pr=32, two=2),
                            in0=G[:, j, 0:B * C].rearrange(
                                "p (x pr two) -> p x pr two", pr=32, two=2),
                            in1=e_rep[:, j, :, :].rearrange(
                                "p x (u two) -> p x u two", u=1)
                                .to_broadcast([128, B * H, 32, 2]),
                            op=mybir.AluOpType.mult)
                        mk = mask_sb[:, c * 128:(c + 1) * 128]
                        first, last = (c == 0), (c == NC_E - 1)
                        nc.tensor.matmul(out=out_p[:, 0:512], lhsT=mk,
                                         rhs=G[:, j, 0:512],
                                         start=first, stop=last)
                        nc.tensor.matmul(out=out_p[:, 512:1024], lhsT=mk,
                                         rhs=G[:, j, 512:1024],
                                         start=first, stop=last)
                        nc.tensor.matmul(
                            out=den_p[:], lhsT=mk,
                            rhs=e_rep[:, j, :, :].rearrange(
                                "p x two -> p (x two)"),
                            start=first, stop=last)
                dsb = small.tile([128, B * H], F32, tag="d")
                nc.vector.tensor_scalar_add(
                    dsb[:],
                    den_p[:].rearrange("p (x two) -> p x two", two=2)[:, :, 0],
                    1e-16)
                rec = small.tile([128, B * H], F32, tag="r")
                nc.vector.reciprocal(rec[:], dsb[:])
                xo = sb.tile([128, B * C], out_dt, tag=f"xo{layer}",
                             name=f"xo{layer}")
                nc.vector.tensor_tensor(
                    out=xo[:].rearrange("p (x d) -> p x d", d=D),
                    in0=out_p[:].rearrange("p (x d) -> p x d", d=D),
                    in1=rec[:].rearrange("p (x u) -> p x u", u=1)
                        .to_broadcast([128, B * H, D]),
                    op=mybir.AluOpType.mult)
                return xo

            x1 = edge_loop(table1, at1, 0, BF16)
            if debug:
                nc.sync.dma_start(out=dbg["dbg_x1"][:], in_=x1[:])
                nc.sync.dma_start(out=dbg["dbg_tbl"][:], in_=table1[:])

            # ---- x1^T (local), AllGather, full x1^T
            x1T_loc = sb.tile([128, 2, B * TPC], BF16)
            for b in range(B):
                for ch in range(2):
                    pt = ps_t.tile([128, 128], BF16, space="PSUM", tag="pt",
                                   name="pt")
                    nc.tensor.transpose(
                        out=pt[:],
                        in_=x1[:, b * C + ch * 128: b * C + (ch + 1) * 128],
                        identity=ident[:])
                    nc.scalar.copy(
                        out=x1T_loc[:, ch, b * 128:(b + 1) * 128], in_=pt[:])
            ag_in = dram.tile([128, 2 * B * TPC], BF16, tag="agin",
                              name="agin")
            ag_out = dram.tile([N, 2 * B * TPC], BF16, addr_space="Shared",
                               tag="agout", name="agout")
            nc.sync.dma_start(
                out=ag_in[:].rearrange("p (ch n) -> p ch n", ch=2),
                in_=x1T_loc[:])
            nc.gpsimd.collective_compute(
                "AllGather", mybir.AluOpType.bypass,
                replica_groups=[list(range(NC))],
                ins=[ag_in.opt()], outs=[ag_out.opt()])
            x1T_full = sb.tile([128, 2, B, N], BF16)
            for k in range(NC):
                nc.sync.dma_start(
                    out=x1T_full[:, :, :, k * 128:(k + 1) * 128],
                    in_=ag_out[k * 128:(k + 1) * 128, :].rearrange(
                        "p (ch b t) -> p ch b t", ch=2, b=B))

            table2 = build_table(
                lambda ch, b, g: x1T_full[:, ch, b, g * 128:(g + 1) * 128],
                "wa2", 2)
            at2 = at_local(
                lambda ch, b: x1T_loc[:, ch, b * 128:(b + 1) * 128], "wa2", 2)

            x2 = edge_loop(table2, at2, 1, F32)
            nc.sync.dma_start(out=y_out[:], in_=x2[:])

    nc.compile()
    return nc


_CACHE: dict = {}


def _get_program(NC_E: int, debug: bool = False):
    key = (NC_E, debug)
    if key not in _CACHE:
        _CACHE[key] = _build(NC_E, debug)
    return _CACHE[key]


def kernel(debug=False, trace=False, **inputs):
    in_maps, NC_E = _prep(**inputs)
    nc = _get_program(NC_E, debug)
    res = run_bass_kernel_spmd(nc, in_maps, core_ids=list(range(NC)),
                               trace=trace)
    y = np.concatenate([res.results[k]["y"] for k in range(NC)], axis=0)
    out = y.reshape(N, B, C)
    if debug or trace:
        return out, res
    return out
